# revision 6
# baseline (speedup 1.0000x reference)
"""CQVAE loss kernel for Trainium2, data-parallel over batch on 8 NeuronCores.

loss = kld(qy) + mse(gather(rzs), zs[:, :Sg]) + bias(best, best_gt)
       + bias(gather(pts), gts)
where bias(p, g) = mse(p, g) + 10 * mse(p[..., MARK, :], g[..., MARK, :]).

Each core handles 16 of the 128 batches.  The rzs and pts rows share the
same gather indices, so the host interleaves them into one combined table
([rzs_row | pts_row] = 1260 floats) and a single indirect DMA per batch
fetches both.  Squared-difference sums are reduced per partition on the
vector/scalar engines and a ones-matmul folds partitions.  Per-core
partial sums are combined on host.
"""

import sys

import numpy as np

try:
    import concourse  # noqa: F401
except ImportError:  # pragma: no cover
    sys.path.insert(0, "/opt/trn_rl_repo")

import concourse.bass as bass
import concourse.mybir as mybir
import concourse.tile as tile
from concourse import bacc
from concourse.bass_utils import run_bass_kernel_spmd

F32 = mybir.dt.float32
I32 = mybir.dt.int32
AX = mybir.AxisListType
OP = mybir.AluOpType
ACTF = mybir.ActivationFunctionType

NCORES = 8
B, S, SG, D, P, V = 128, 256, 128, 1024, 118, 64
BL = B // NCORES  # batches per core
P2 = 2 * P  # 236 floats per point-row
CW = D + P2  # combined gather row width (1260)
MARK = (0, 29, 88, 117)
EPS = 1e-20
ALPHA = 10.0

KA = 2  # batches per gather/compute group
NG = BL // KA  # 8 groups
BLB = B // NCORES  # best rows per core

# stats columns in the [128, 16] allstats tile:
#  0=bias_sq 1=bias_mark_sq 2=kld_num 3=best_sq 4=best_mark_sq 5..7 unused
#  8..15 = per-group ae_sq accumulators
NSTAT = 16

_module = None
last_results = None  # BassKernelResults of the most recent run (for profiling)


def _build_module():
    nc = bacc.Bacc()

    zs = nc.dram_tensor("zs", [BL * S, D], F32, kind="ExternalInput")
    comb = nc.dram_tensor("comb", [BL * S, CW], F32, kind="ExternalInput")
    gts = nc.dram_tensor("gts", [BL * SG, P2], F32, kind="ExternalInput")
    qy = nc.dram_tensor("qy", [BL * S, V], F32, kind="ExternalInput")
    best = nc.dram_tensor("best", [BLB, P2], F32, kind="ExternalInput")
    best_gt = nc.dram_tensor("best_gt", [BLB, P2], F32, kind="ExternalInput")
    # idx[i, b] = b*S + mapping[b, i]: flat row into the per-core comb shard
    idx = nc.dram_tensor("idx", [SG, BL], I32, kind="ExternalInput")
    out = nc.dram_tensor("out", [1, NSTAT], F32, kind="ExternalOutput")

    QCOLS = BL * S * V // 128  # 2048
    QN = BL * S // 128  # 32 rows per partition

    with tile.TileContext(nc) as tc:
        with (
            tc.tile_pool(name="sb", bufs=4) as sb,
            tc.tile_pool(name="cst", bufs=1) as cst,
            tc.tile_pool(name="ps", bufs=1, space="PSUM") as ps,
        ):
            idx_t = cst.tile([SG, BL], I32)
            nc.sync.dma_start(idx_t[:], idx[:])

            stats = cst.tile([128, NSTAT], F32)
            nc.vector.memset(stats[:], 0.0)
            acc_b = cst.tile([128, KA * P2], F32)
            nc.vector.memset(acc_b[:], 0.0)

            # --- KLD: sum q * (log(q + eps) - log(1/V)) via log(V*q + V*eps) ---
            qy_t = cst.tile([128, QCOLS], F32)
            nc.scalar.dma_start(
                qy_t[:].rearrange("p (n v) -> p n v", v=V),
                qy[:].rearrange("(p n) v -> p n v", n=QN),
            )
            lg = cst.tile([128, QCOLS], F32)
            ebias = cst.tile([128, 1], F32)
            nc.vector.memset(ebias[:], float(V) * EPS)
            nc.scalar.activation(lg[:], qy_t[:], ACTF.Ln, bias=ebias[:], scale=float(V))
            nc.vector.scalar_tensor_tensor(
                out=lg[:],
                in0=lg[:],
                scalar=0.0,
                in1=qy_t[:],
                op0=OP.subtract,
                op1=OP.mult,
                accum_out=stats[:, 2:3],
            )

            # --- BEST: per-core shard [BLB, P2] ---
            bt = cst.tile([BLB, P2], F32)
            nc.scalar.dma_start(bt[:], best[:])
            bgt = cst.tile([BLB, P2], F32)
            nc.scalar.dma_start(bgt[:], best_gt[:])
            nc.vector.tensor_sub(bt[:], bt[:], bgt[:])
            nc.vector.tensor_mul(bt[:], bt[:], bt[:])
            nc.vector.reduce_sum(out=stats[:BLB, 3:4], in_=bt[:], axis=AX.X)
            bm4 = cst.tile([BLB, 4], F32)
            for j, m in enumerate(MARK):
                nc.vector.reduce_sum(
                    out=bm4[:, j : j + 1], in_=bt[:, 2 * m : 2 * m + 2], axis=AX.X
                )
            nc.vector.reduce_sum(out=stats[:BLB, 4:5], in_=bm4[:], axis=AX.X)

            # --- combined AE + BIAS groups (KA batches each) ---
            zs_r = zs[:].rearrange("(b s) d -> s b d", s=S)
            gts_r = gts[:].rearrange("(b p) c -> p b c", p=SG)
            for g in range(NG):
                b0 = g * KA
                cg = sb.tile([128, KA * CW], F32, tag="cg")
                for k in range(KA):
                    nc.gpsimd.indirect_dma_start(
                        out=cg[:, (k * CW) : ((k + 1) * CW)],
                        out_offset=None,
                        in_=comb[:],
                        in_offset=bass.IndirectOffsetOnAxis(
                            ap=idx_t[:, b0 + k : b0 + k + 1], axis=0
                        ),
                    )
                cg3 = cg[:].rearrange("p (k c) -> p k c", c=CW)
                # AE part
                zt = sb.tile([128, KA * D], F32, tag="zt")
                nc.sync.dma_start(
                    zt[:].rearrange("p (k d) -> p k d", d=D),
                    zs_r[0:SG, b0 : b0 + KA, :],
                )
                nc.vector.tensor_sub(
                    cg3[:, :, 0:D], cg3[:, :, 0:D],
                    zt[:].rearrange("p (k d) -> p k d", d=D),
                )
                nc.scalar.activation(
                    cg3[:, :, 0:D],
                    cg3[:, :, 0:D],
                    ACTF.Square,
                    accum_out=stats[:, 8 + g : 9 + g],
                )
                # BIAS part
                gt2 = sb.tile([128, KA * P2], F32, tag="gt2")
                nc.scalar.dma_start(
                    gt2[:].rearrange("p (k c) -> p k c", c=P2),
                    gts_r[:, b0 : b0 + KA, :],
                )
                nc.vector.tensor_sub(
                    cg3[:, :, D:CW], cg3[:, :, D:CW],
                    gt2[:].rearrange("p (k c) -> p k c", c=P2),
                )
                nc.scalar.activation(cg3[:, :, D:CW], cg3[:, :, D:CW], ACTF.Square)
                nc.vector.tensor_add(
                    acc_b[:].rearrange("p (k c) -> p k c", c=P2),
                    acc_b[:].rearrange("p (k c) -> p k c", c=P2),
                    cg3[:, :, D:CW],
                )

            # --- fold bias accumulator into stats ---
            nc.vector.reduce_sum(out=stats[:, 0:1], in_=acc_b[:], axis=AX.X)
            bk4 = cst.tile([128, 4], F32)
            acc_b3 = acc_b[:].rearrange("p (k c) -> p k c", c=P2)
            for j, m in enumerate(MARK):
                nc.vector.reduce_sum(
                    out=bk4[:, j : j + 1],
                    in_=acc_b3[:, :, 2 * m : 2 * m + 2],
                    axis=AX.XY,
                )
            nc.vector.reduce_sum(out=stats[:, 1:2], in_=bk4[:], axis=AX.X)

            # --- partition fold: ones^T @ stats -> [1, NSTAT] ---
            ones = cst.tile([128, 1], F32)
            nc.vector.memset(ones[:], 1.0)
            pst = ps.tile([1, NSTAT], F32)
            nc.tensor.matmul(
                out=pst[:], lhsT=ones[:], rhs=stats[:], start=True, stop=True
            )
            res = cst.tile([1, NSTAT], F32)
            nc.vector.tensor_copy(res[:], pst[:])
            nc.sync.dma_start(out[:], res[:])

    nc.compile()
    return nc


def kernel(
    zs, rzs, pts, best, qy, gts, best_gt, mapping, vector_dims, **trace_kwargs
):
    global _module, last_results
    vd = int(np.asarray(vector_dims))
    assert vd == V, f"kernel compiled for vector_dims={V}, got {vd}"

    if _module is None:
        _module = _build_module()

    zs = np.asarray(zs, dtype=np.float32)
    rzs = np.asarray(rzs, dtype=np.float32)
    pts = np.asarray(pts, dtype=np.float32).reshape(B, S, P2)
    gts = np.asarray(gts, dtype=np.float32)
    qy = np.asarray(qy, dtype=np.float32)
    mapping = np.asarray(mapping).astype(np.int32)
    best2 = np.asarray(best, dtype=np.float32).reshape(B, P2)
    bgt2 = np.asarray(best_gt, dtype=np.float32).reshape(B, P2)

    # interleave rzs|pts rows so one gather fetches both
    comb = np.concatenate([rzs, pts], axis=2)  # [B, S, CW]

    base = (np.arange(BL, dtype=np.int32) * S)[:, None]
    in_maps = []
    for c in range(NCORES):
        sl = slice(c * BL, (c + 1) * BL)
        in_maps.append(
            {
                "zs": zs[sl].reshape(BL * S, D),
                "comb": comb[sl].reshape(BL * S, CW),
                "gts": gts[sl].reshape(BL * SG, P2),
                "qy": qy[sl].reshape(BL * S, V),
                "best": np.ascontiguousarray(best2[sl]),
                "best_gt": np.ascontiguousarray(bgt2[sl]),
                "idx": np.ascontiguousarray((mapping[sl] + base).T),
            }
        )

    last_results = run_bass_kernel_spmd(
        _module, in_maps, list(range(NCORES)), **trace_kwargs
    )
    parts = np.stack(
        [
            np.asarray(r["out"], dtype=np.float64).reshape(NSTAT)
            for r in last_results.results
        ]
    )
    tot = parts.sum(axis=0)

    ae_loss = tot[8:16].sum() / (B * SG * D)
    bias_loss = tot[0] / (B * SG * P2) + ALPHA * tot[1] / (B * SG * 2 * len(MARK))
    kld_loss = tot[2] / (B * S)
    best_mse = tot[3] / (B * P2) + ALPHA * tot[4] / (B * 2 * len(MARK))

    return np.array(kld_loss + ae_loss + best_mse + bias_loss, dtype=np.float32)
